# revision 1
# baseline (speedup 1.0000x reference)
"""Trainium2 Bass kernel for nn_AttentionScore (sparse local attention scores).

Reference computation (B=4, C=64, N=16384, S=16):
    tmp   = xyz[:, :, :, None] - neighbor_xyz            # [B,3,N,S]
    pos   = concat([tmp, ||tmp||], axis=1)               # [B,4,N,S]
    k     = Wk @ (neighbor_points + Wpos @ pos + bpos)   # [B,C,N,S]
    attn  = softmax_s((points*scale) . k)                # [B,N,S]

Softmax over s is shift-invariant, so every term constant in s drops out:
    attn[m,s] ~ sum_c qW[c,m]*np[c,m,s] + sum_j qp[j,m]*tmp[j,m,s] + qp3[m]*||tmp||
with qW = (scale*Wk)^T @ points, qp = Wpos^T @ qW (bpos and the xyz.qp dot cancel).

Sharding: N split contiguously across 8 cores (no communication needed).
m = b*2048 + n_local in [0, 8192) per core, split in halves h = m // 4096.

Main-term dataflow per core:
  - np staged as [128 part = (h,c), (mm,s)] tiles; DVE multiplies by qW
    broadcast over s; TensorE reduces the 64 c-partitions per half with a
    block-ones [128,2] matmul (4x col-tiled into PSUM partitions 32j+h);
    ScalarE copies PSUM->SBUF; a partition-scatter SBUF->SBUF DMA lands
    results in the softmax layout [p = m//64, (m%64)*16+s].
"""

import os
import sys

sys.path.insert(0, "/opt/trn_rl_repo")

import numpy as np

import concourse.bass as bass
import concourse.bacc as bacc
import concourse.tile as tile
from concourse import mybir
from concourse.bass_utils import run_bass_kernel_spmd

F32 = mybir.dt.float32
F32R = mybir.dt.float32r
BF16 = mybir.dt.bfloat16
AF = mybir.ActivationFunctionType
AX = mybir.AxisListType
OP = mybir.AluOpType

B, C, N, S = 4, 64, 16384, 16
NCORES = 8
NL = N // NCORES            # 2048 points per core
M = B * NL                  # 8192 (b, n) rows per core
MH = M // 2                 # 4096 rows per half
MB = 256                    # mm per supertile (per half)
NT = MH // MB               # 16 supertiles
SCALE = float(C) ** -0.5

# float32r streams the moving operand at 1 col/cycle (vs 4 for fp32) and is
# more precise than the fp32 emulation path. Used for the big channel
# reduction only; small matmuls (qW, qp) stay fp32.
USE_F32R_REDUCE = True


def _body(tc):
    nc = tc.nc
    dma = nc.sync.dma_start

    NP = nc.dram_tensor("NP", [128, MH * S], F32, kind="ExternalInput").ap()
    NX = nc.dram_tensor("NX", [128, (M // 128) * 3 * S], F32, kind="ExternalInput").ap()
    XYZ = nc.dram_tensor("XYZ", [128, (M // 128) * 3], F32, kind="ExternalInput").ap()
    P = nc.dram_tensor("P", [C, M], F32, kind="ExternalInput").ap()
    WK = nc.dram_tensor("WK", [C, C], F32, kind="ExternalInput").ap()
    WKT = nc.dram_tensor("WKT", [C, C], F32, kind="ExternalInput").ap()
    WP = nc.dram_tensor("WP", [C, 4], F32, kind="ExternalInput").ap()
    OUT = nc.dram_tensor("OUT", [128, (M // 128) * S], F32, kind="ExternalOutput").ap()

    RDT = F32R if USE_F32R_REDUCE else F32

    with (
        tc.tile_pool(name="const", bufs=1) as cp,
        tc.tile_pool(name="w3072", bufs=2) as p3072,
        tc.tile_pool(name="w1024", bufs=3) as p1024,
        tc.tile_pool(name="small", bufs=1) as sp,
        # main-loop pools open up-front so NP prefetch DMAs have their SBUF
        # space from the start and fully overlap phase 1/2
        tc.tile_pool(name="npt", bufs=3) as npp,
        tc.tile_pool(name="prod", bufs=2) as prp,
        tc.tile_pool(name="sc", bufs=2) as scp,
        tc.tile_pool(name="psm", bufs=2, space="PSUM") as psm,
    ):
        # ---- constant loads ----
        wk = cp.tile([C, C], F32)
        dma(wk[:], WK)
        wkt = cp.tile([C, C], F32)
        dma(wkt[:], WKT)
        wp = cp.tile([C, 4], F32)
        dma(wp[:], WP)
        nxt = cp.tile([128, 64 * 3 * S], F32)
        dma(nxt[:], NX)
        xyzt = cp.tile([128, 64 * 3], F32)
        dma(xyzt[:], XYZ)

        wks = sp.tile([C, C], F32)
        nc.vector.tensor_scalar_mul(wks[:], wk[:], SCALE)
        wkts = sp.tile([C, C], F32)
        nc.vector.tensor_scalar_mul(wkts[:], wkt[:], SCALE)

        # Per-chunk half-selectors: lhsT for chunk k is hs[:, k*16:(k+1)*16],
        # whose column h*8+k is 1 on the h-half partitions. The 8 chunk
        # matmuls accumulate into one [16, 512] PSUM tile with chunk k's
        # half-h sums landing on row h*8+k (other rows accumulate zeros).
        # Built in f32 and copied with an f32r-rounding DVE op so the
        # fp32r matmul sees a properly "rounded" producer.
        hs0 = sp.tile([128, 16 * 8], F32)
        nc.vector.memset(hs0[:], 0.0)
        for k in range(8):
            nc.vector.memset(hs0[0:64, k * 16 + k:k * 16 + k + 1], 1.0)
            nc.vector.memset(hs0[64:128, k * 16 + 8 + k:k * 16 + 8 + k + 1], 1.0)
        hs = sp.tile([128, 16 * 8], RDT)
        nc.vector.tensor_copy(hs[:], hs0[:])

        qw = cp.tile([128, MH], F32)      # row h*64+c holds qW[c, h*MH + mm]
        qpt = cp.tile([128, 4 * 64], F32)  # row p, col j*64+mi: qp[j, p*64+mi]
        attn1 = cp.tile([128, 64 * S], F32)
        attn2 = cp.tile([128, 64 * S], F32)

        # ---- phase 1: qW / qp via bf16 Karatsuba on the PE ----
        # X @ Y ~= Xh@Yh + Xh@Yl + Xl@Yh with h/l the bf16 split; ~2^-18
        # relative error at 1 cycle/col (vs 4 for the fp32 emulation).
        # Chunked q keeps SBUF small so NP prefetch overlaps phase 1; (h0,h1)
        # chunk pairs emit in cc order so early supertiles unblock first.
        CH = 512
        NC1 = M // CH
        with (
            tc.tile_pool(name="qchunk", bufs=2) as qcp,
            tc.tile_pool(name="qps_p", bufs=2) as qpsp,
            tc.tile_pool(name="psq", bufs=2, space="PSUM") as psq,
            tc.tile_pool(name="psp", bufs=2, space="PSUM") as psp,
            tc.tile_pool(name="psw", bufs=1, space="PSUM") as psw,
        ):
            # Wkp[c, j] = sum_c' (scale*Wk)[c, c'] Wpos[c', j]  (fp32, tiny)
            pwkp = psw.tile([C, 4], F32)
            nc.tensor.matmul(pwkp[:], lhsT=wkts[:], rhs=wp[:], start=True, stop=True)
            wkp = sp.tile([C, 4], F32)
            nc.scalar.copy(wkp[:], pwkp[:])

            # Zero-padded qW weights: block h is [64, 128] with cols
            # h*64..h*64+64 = scale*Wk, so out rows h*64.. hold qW and every
            # PSUM partition is written. Split into bf16 hi/lo.
            wkh0 = sp.tile([C, 2 * 128], F32)
            nc.vector.memset(wkh0[:], 0.0)
            nc.vector.tensor_copy(wkh0[:, 0:64], wks[:])
            nc.vector.tensor_copy(wkh0[:, 192:256], wks[:])
            whh = sp.tile([C, 2 * 128], BF16)
            nc.vector.tensor_copy(whh[:], wkh0[:])
            whl0 = sp.tile([C, 2 * 128], F32)
            nc.vector.tensor_sub(whl0[:], wkh0[:], whh[:])
            whl = sp.tile([C, 2 * 128], BF16)
            nc.vector.tensor_copy(whl[:], whl0[:])

            wkph = sp.tile([C, 4], BF16)
            nc.vector.tensor_copy(wkph[:], wkp[:])
            wkpl0 = sp.tile([C, 4], F32)
            nc.vector.tensor_sub(wkpl0[:], wkp[:], wkph[:])
            wkpl = sp.tile([C, 4], BF16)
            nc.vector.tensor_copy(wkpl[:], wkpl0[:])

            # (h0, h1) chunk pairs in cc order so qw columns needed by the
            # first supertiles are produced first.
            qps_tiles = {}
            qps_fill = {}
            for cc in range(NC1 // 2):
              for h in range(2):
                t = h * (NC1 // 2) + cc
                rows = slice(h * 64, h * 64 + 64)
                wsl = slice(h * 128, (h + 1) * 128)

                qf = qcp.tile([C, CH], F32, tag="qf")
                # first pair rides the (empty) Sync queue ahead of NP tile 0
                # so supertile 0's qW dependency clears early; later chunks
                # go through SWDGE to keep Sync free for NP prefetch.
                if cc == 0:
                    dma(qf[:], P[:, t * CH:(t + 1) * CH])
                else:
                    nc.gpsimd.dma_start(qf[:], P[:, t * CH:(t + 1) * CH])
                qhh = qcp.tile([C, CH], BF16, tag="qhh")
                nc.scalar.copy(qhh[:], qf[:])
                qll = qcp.tile([C, CH], BF16, tag="qll")
                nc.vector.tensor_sub(qll[:], qf[:], qhh[:])

                cc2 = (t % (NC1 // 2)) * CH

                # qW[c', m] = sum_c (scale*Wk)[c, c'] q[c, m]
                pq = psq.tile([128, 512], F32)
                nc.tensor.matmul(pq[:], lhsT=whh[:, wsl], rhs=qhh[:], start=True, stop=False)
                nc.tensor.matmul(pq[:], lhsT=whh[:, wsl], rhs=qll[:], start=False, stop=False)
                nc.tensor.matmul(pq[:], lhsT=whl[:, wsl], rhs=qhh[:], start=False, stop=True)
                nc.scalar.copy(qw[rows, cc2:cc2 + CH], pq[rows, :])

                # qp[j, m] = sum_c Wkp[c, j] q[c, m]
                pp = psp.tile([4, 512], F32)
                nc.tensor.matmul(pp[:], lhsT=wkph[:], rhs=qhh[:], start=True, stop=False)
                nc.tensor.matmul(pp[:], lhsT=wkph[:], rhs=qll[:], start=False, stop=False)
                nc.tensor.matmul(pp[:], lhsT=wkpl[:], rhs=qhh[:], start=False, stop=True)

                g, gi = divmod(t, 4)
                if g not in qps_tiles:
                    qps_tiles[g] = qpsp.tile([4, 2048], F32, name="qps", tag="qps")
                    qps_fill[g] = 0
                qps = qps_tiles[g]
                nc.scalar.copy(qps[:, gi * 512:(gi + 1) * 512], pp[:])
                qps_fill[g] += 1
                if qps_fill[g] == 4:
                    # scatter qp group into softmax layout: qpt[p, j*64+mi]
                    for j in range(4):
                        nc.gpsimd.dma_start(
                            qpt[g * 32:(g + 1) * 32, j * 64:(j + 1) * 64],
                            qps[j:j + 1, :],
                        )
                    del qps_tiles[g]

        # ---- phase 2: positional term (whole core at once) ----
        # tmp[p, mi, j, s] = xyz[j, m] - nx[j, m, s]
        nx3 = nxt[:].rearrange("p (mi j s) -> p mi j s", mi=64, j=3, s=S)
        xyzb = (
            xyzt[:]
            .rearrange("p (mi j one) -> p mi j one", mi=64, j=3, one=1)
            .broadcast_to((128, 64, 3, S))
        )
        tmp = p3072.tile([128, 64 * 3 * S], F32, tag="big")
        tmp3 = tmp[:].rearrange("p (mi j s) -> p mi j s", mi=64, j=3, s=S)
        nc.vector.tensor_sub(tmp3, xyzb, nx3)

        sq = p3072.tile([128, 64 * 3 * S], F32, tag="big")
        nc.scalar.square(sq[:], tmp[:])

        norm2 = p1024.tile([128, 64 * S], F32, tag="w1k")
        nc.vector.reduce_sum(
            norm2[:].rearrange("p (mi s) -> p mi s", mi=64),
            sq[:].rearrange("p (mi j s) -> p mi s j", mi=64, j=3, s=S),
            axis=AX.X,
        )
        norm = p1024.tile([128, 64 * S], F32, tag="w1k")
        nc.scalar.sqrt(norm[:], norm2[:])

        # u = sum_j qp[j]*tmp[j]
        qptb3 = (
            qpt[:]
            .rearrange("p (j mi one) -> p mi j one", j=4, mi=64, one=1)[:, :, 0:3, :]
            .broadcast_to((128, 64, 3, S))
        )
        uw = p3072.tile([128, 64 * 3 * S], F32, tag="big")
        uw3 = uw[:].rearrange("p (mi j s) -> p mi j s", mi=64, j=3, s=S)
        nc.vector.tensor_mul(uw3, tmp3, qptb3)
        u = p1024.tile([128, 64 * S], F32, tag="w1k")
        nc.vector.reduce_sum(
            u[:].rearrange("p (mi s) -> p mi s", mi=64),
            uw[:].rearrange("p (mi j s) -> p mi s j", mi=64, j=3, s=S),
            axis=AX.X,
        )

        # attn2 = u + qp3 * norm
        qp3b = (
            qpt[:, 192:256]
            .rearrange("p (mi one) -> p mi one", one=1)
            .broadcast_to((128, 64, S))
        )
        a2 = p1024.tile([128, 64 * S], F32, tag="w1k")
        a23 = a2[:].rearrange("p (mi s) -> p mi s", mi=64)
        nc.vector.tensor_mul(a23, norm[:].rearrange("p (mi s) -> p mi s", mi=64), qp3b)
        nc.vector.tensor_add(attn2[:], a2[:], u[:])

        # ---- phase 3: main term supertiles ----
        if True:
            for t in range(NT):
                npt = npp.tile([128, MB * S], F32)
                dma(npt[:], NP[:, t * MB * S:(t + 1) * MB * S])

                prod = prp.tile([128, MB * S], RDT)
                qwb = (
                    qw[:, t * MB:(t + 1) * MB]
                    .rearrange("p (mm one) -> p mm one", one=1)
                    .broadcast_to((128, MB, S))
                )
                nc.vector.tensor_mul(
                    prod[:].rearrange("p (mm s) -> p mm s", s=S),
                    npt[:].rearrange("p (mm s) -> p mm s", s=S),
                    qwb,
                )

                ps = psm.tile([16, 512], F32)
                for k in range(8):
                    nc.tensor.matmul(
                        ps[:],
                        lhsT=hs[:, k * 16:(k + 1) * 16],
                        rhs=prod[:, k * 512:(k + 1) * 512],
                        start=(k == 0),
                        stop=(k == 7),
                    )
                sc = scp.tile([16, 512], F32)
                nc.scalar.copy(sc[:], ps[:])
                # row h*8+k holds chunk k / half h; lands at attn1 partition
                # h*64 + t*4 + k//2, cols (k%2)*512 + i*16 + s. Issued from
                # GPSIMD (SWDGE) so their waits don't stall the Sync queue
                # that prefetches NP tiles.
                for h in range(2):
                    nc.gpsimd.dma_start(
                        attn1[h * 64 + t * 4:h * 64 + t * 4 + 4, :].rearrange(
                            "p (k1 f) -> p k1 f", k1=2
                        ),
                        sc[h * 8:(h + 1) * 8, :],
                    )

        # ---- phase 4: softmax over s ----
        attn = p1024.tile([128, 64 * S], F32, tag="w1k")
        nc.vector.tensor_add(attn[:], attn1[:], attn2[:])
        at3 = attn[:].rearrange("p (mi s) -> p mi s", mi=64)

        mx = sp.tile([128, 64], F32)
        nc.vector.reduce_max(mx[:], at3, axis=AX.X)
        mxb = mx[:].rearrange("p (mi one) -> p mi one", one=1).broadcast_to((128, 64, S))
        xs = p1024.tile([128, 64 * S], F32, tag="w1k")
        nc.vector.tensor_sub(xs[:].rearrange("p (mi s) -> p mi s", mi=64), at3, mxb)

        e = p1024.tile([128, 64 * S], F32, tag="w1k")
        nc.scalar.activation(e[:], xs[:], AF.Exp)

        se = sp.tile([128, 64], F32)
        nc.vector.reduce_sum(se[:], e[:].rearrange("p (mi s) -> p mi s", mi=64), axis=AX.X)
        rse = sp.tile([128, 64], F32)
        nc.vector.reciprocal(rse[:], se[:])

        o = p1024.tile([128, 64 * S], F32, tag="w1k")
        rb = rse[:].rearrange("p (mi one) -> p mi one", one=1).broadcast_to((128, 64, S))
        nc.vector.tensor_mul(
            o[:].rearrange("p (mi s) -> p mi s", mi=64),
            e[:].rearrange("p (mi s) -> p mi s", mi=64),
            rb,
        )
        dma(OUT, o[:])


_NC_CACHE = None


def build_nc():
    global _NC_CACHE
    if _NC_CACHE is None:
        nc = bacc.Bacc(trn_type="TRN2", target_bir_lowering=False, debug=False)
        with tile.TileContext(nc) as tc:
            _body(tc)
        nc.compile()
        _NC_CACHE = nc
    return _NC_CACHE


def make_in_maps(xyz, neighbor_xyz, points, neighbor_points, Wk, Wpos, bpos):
    """Slice + relayout full inputs into the 8 per-core input maps."""
    xyz = np.asarray(xyz, dtype=np.float32)
    neighbor_xyz = np.asarray(neighbor_xyz, dtype=np.float32)
    points = np.asarray(points, dtype=np.float32)
    neighbor_points = np.asarray(neighbor_points, dtype=np.float32)
    Wk = np.ascontiguousarray(np.asarray(Wk, dtype=np.float32))
    WkT = np.ascontiguousarray(Wk.T)
    Wp = np.ascontiguousarray(np.asarray(Wpos, dtype=np.float32))

    in_maps = []
    for i in range(NCORES):
        nsl = slice(i * NL, (i + 1) * NL)
        # np: [B,C,nl,S] -> [c, m, s] -> [h, c, mm, s] -> [128, MH*S]
        npc = neighbor_points[:, :, nsl, :].transpose(1, 0, 2, 3).reshape(C, M, S)
        npc = (
            npc.reshape(C, 2, MH, S).transpose(1, 0, 2, 3).reshape(128, MH * S)
        )
        # nx: [B,3,nl,S] -> [m, j, s] -> [128, 64*3*S]
        nxc = (
            neighbor_xyz[:, :, nsl, :]
            .transpose(1, 0, 2, 3)
            .reshape(3, M, S)
            .transpose(1, 0, 2)
            .reshape(128, 64 * 3 * S)
        )
        # xyz: [B,3,nl] -> [m, j] -> [128, 192]
        xc = (
            xyz[:, :, nsl]
            .transpose(1, 0, 2)
            .reshape(3, M)
            .T.reshape(128, 64 * 3)
        )
        # points: [B,C,nl] -> [c, m]
        pc = points[:, :, nsl].transpose(1, 0, 2).reshape(C, M)
        in_maps.append(
            {
                "NP": np.ascontiguousarray(npc),
                "NX": np.ascontiguousarray(nxc),
                "XYZ": np.ascontiguousarray(xc),
                "P": np.ascontiguousarray(pc),
                "WK": Wk,
                "WKT": WkT,
                "WP": Wp,
            }
        )
    return in_maps


def assemble_output(results):
    """Per-core OUT [128, 64*S] -> full [B, N, S]."""
    out = np.empty((B, N, S), dtype=np.float32)
    for i in range(NCORES):
        oc = np.asarray(results[i]["OUT"]).reshape(M, S)  # m = p*64+mi row-major
        out[:, i * NL:(i + 1) * NL, :] = oc.reshape(B, NL, S)
    return out


def run_cores(in_maps, trace=False, trace_kwargs=None):
    nc = build_nc()
    return run_bass_kernel_spmd(
        nc,
        in_maps,
        core_ids=list(range(NCORES)),
        trace=trace,
        **(trace_kwargs or {}),
    )


def kernel(xyz, neighbor_xyz, points, neighbor_points, Wk, Wpos, bpos):
    in_maps = make_in_maps(
        xyz, neighbor_xyz, points, neighbor_points, Wk, Wpos, bpos
    )
    res = run_cores(in_maps, trace=False)
    return assemble_output(res.results)



# revision 9
# speedup vs baseline: 1.8088x; 1.8088x over previous
"""Trainium2 Bass kernel for nn_AttentionScore (sparse local attention scores).

Reference computation (B=4, C=64, N=16384, S=16):
    tmp   = xyz[:, :, :, None] - neighbor_xyz            # [B,3,N,S]
    pos   = concat([tmp, ||tmp||], axis=1)               # [B,4,N,S]
    k     = Wk @ (neighbor_points + Wpos @ pos + bpos)   # [B,C,N,S]
    attn  = softmax_s((points*scale) . k)                # [B,N,S]

Softmax over s is shift-invariant, so every term constant in s drops out:
    attn[m,s] ~ sum_c qW[c,m]*np[c,m,s] + sum_j qp[j,m]*tmp[j,m,s] + qp3[m]*||tmp||
with qW = (scale*Wk)^T @ points, qp = Wpos^T @ qW (bpos and the xyz.qp dot cancel).

Sharding: N split contiguously across 8 cores (no communication needed).
m = b*2048 + n_local in [0, 8192) per core.

v2: all big HBM streams staged as bf16 (halves DMA), single-pass bf16
matmuls, DVE ops arranged for the 2x_1p packed mode (bf16 + innermost
unit-stride pairs; the per-m qW broadcast over s is made packable by
storing qW duplicated into adjacent pairs).

Layouts per core (M = 8192 rows, supertile t covers m in [1024t, 1024(t+1))):
  NP  [128=(d,c), (t, mm512, s)] bf16, d = (m//512)%2, mm = m%512
  NX  [128=m//64, (mi, j, s)] bf16
  XYZ2[128=m//64, (mi, j, 2dup)] bf16 (pre-duplicated pairs)
  P   [64=c, m] bf16
  OUT [128=m//64, (mi, s)] f32

Main term per supertile: DVE multiplies np by qW (bf16 2x mode, qW stored
pair-duplicated), TensorE reduces the 64 c-partitions per d-group with
block-ones selectors (16 matmuls x 512 cols accumulating into one
[32, 512] PSUM tile, row d*16+k = chunk k of group d), ScalarE copies
PSUM->SBUF, and a partition-scatter SBUF->SBUF DMA lands the softmax
layout [p=m//64, (m%64)*16+s].
"""

import os
import sys

sys.path.insert(0, "/opt/trn_rl_repo")

import numpy as np
import ml_dtypes

import concourse.bass as bass
import concourse.bacc as bacc
import concourse.tile as tile
from concourse import mybir
from concourse.bass_utils import run_bass_kernel_spmd

F32 = mybir.dt.float32
BF16 = mybir.dt.bfloat16
AF = mybir.ActivationFunctionType
AX = mybir.AxisListType
OP = mybir.AluOpType

BF = ml_dtypes.bfloat16

B, C, N, S = 4, 64, 16384, 16
NCORES = 8
NL = N // NCORES            # 2048 points per core
M = B * NL                  # 8192 (b, n) rows per core
MB = 512                    # mm per supertile per d-group
ST = M // (2 * MB)          # 8 supertiles, each covering 1024 m
CH = 512                    # phase-1 q chunk (one d-block)
NC1 = M // CH               # 16 chunks
SCALE = float(C) ** -0.5


def _body(tc):
    nc = tc.nc
    dma = nc.sync.dma_start

    NP = nc.dram_tensor("NP", [128, ST * MB * S], BF16, kind="ExternalInput").ap()
    NX = nc.dram_tensor("NX", [128, 64 * 3 * S], BF16, kind="ExternalInput").ap()
    XYZ2 = nc.dram_tensor("XYZ2", [128, 64 * 3 * 2], BF16, kind="ExternalInput").ap()
    P = nc.dram_tensor("P", [C, M], BF16, kind="ExternalInput").ap()
    WK = nc.dram_tensor("WK", [C, C], F32, kind="ExternalInput").ap()
    WKT = nc.dram_tensor("WKT", [C, C], F32, kind="ExternalInput").ap()
    WP = nc.dram_tensor("WP", [C, 4], F32, kind="ExternalInput").ap()
    OUT = nc.dram_tensor("OUT", [128, (M // 128) * S], F32, kind="ExternalOutput").ap()

    with (
        tc.tile_pool(name="const", bufs=1) as cp,
        tc.tile_pool(name="small", bufs=1) as sp,
        tc.tile_pool(name="pchunk", bufs=2) as pcp,
        tc.tile_pool(name="qps_p", bufs=2) as qpsp,
        tc.tile_pool(name="w3072", bufs=1) as p3072,
        tc.tile_pool(name="w1024", bufs=4) as p1024,
        # main loop pools
        tc.tile_pool(name="npt", bufs=4) as npp,
        tc.tile_pool(name="prod", bufs=2) as prp,
        tc.tile_pool(name="sc", bufs=2) as scp,
        tc.tile_pool(name="psq", bufs=2, space="PSUM") as psq,
        tc.tile_pool(name="psw", bufs=1, space="PSUM") as psw,
        tc.tile_pool(name="psm", bufs=4, space="PSUM") as psm,
    ):
        # ---- constant loads (ACT HWDGE queue; NP stream rides Sync) ----
        wk = cp.tile([C, C], F32)
        nc.scalar.dma_start(wk[:], WK)
        wkt = cp.tile([C, C], F32)
        nc.scalar.dma_start(wkt[:], WKT)
        wp = cp.tile([C, 4], F32)
        nc.scalar.dma_start(wp[:], WP)
        pt0 = pcp.tile([C, 4 * CH], BF16, tag="pch")
        nc.scalar.dma_start(pt0[:], P[:, 0:4 * CH])
        xyz2 = cp.tile([128, 64 * 3 * 2], BF16)
        nc.scalar.dma_start(xyz2[:], XYZ2)
        nxt = cp.tile([128, 64 * 3 * S], BF16)
        nc.scalar.dma_start(nxt[:], NX)
        pt1 = pcp.tile([C, 4 * CH], BF16, tag="pch")
        nc.scalar.dma_start(pt1[:], P[:, 4 * CH:8 * CH])
        pts = [pt0, pt1]

        # ---- tiny weight prep ----
        wks = sp.tile([C, C], BF16)
        nc.vector.tensor_scalar_mul(wks[:], wk[:], SCALE)
        wkts = sp.tile([C, C], BF16)
        nc.vector.tensor_scalar_mul(wkts[:], wkt[:], SCALE)
        wpb = sp.tile([C, 4], BF16)
        nc.vector.tensor_copy(wpb[:], wp[:])

        # Wkp[c, j] = sum_c' (scale*Wk)[c, c'] Wpos[c', j]
        pwkp = psw.tile([C, 4], F32)
        nc.tensor.matmul(pwkp[:], lhsT=wkts[:], rhs=wpb[:], start=True, stop=True)
        wkp = sp.tile([C, 4], BF16)
        nc.scalar.copy(wkp[:], pwkp[:])

        # Fused phase-1 weights, one per d-group: [64, 128] where cols
        # d*64..d*64+64 = scale*Wk (-> qW on out rows d*64+c) and cols
        # (1-d)*64..+4 = Wkp (-> qp on out rows (1-d)*64+j).
        whs = []
        for d in range(2):
            wh = sp.tile([C, 128], BF16, name=f"wh{d}", tag=f"wh{d}")
            nc.vector.memset(wh[:], 0.0)
            nc.vector.tensor_copy(wh[:, d * 64:d * 64 + 64], wks[:])
            nc.vector.tensor_copy(wh[:, (1 - d) * 64:(1 - d) * 64 + 4], wkp[:])
            whs.append(wh)

        # Reduce selectors: chunk k of 16 (32 mm each), lhsT window k is
        # hs[:, 32k:32k+32]; col d*16+k = 1 on the d-group partitions.
        hs = sp.tile([128, 16 * 32], BF16)
        nc.vector.memset(hs[:], 0.0)
        for k in range(16):
            nc.vector.memset(hs[0:64, k * 32 + k:k * 32 + k + 1], 1.0)
            nc.vector.memset(hs[64:128, k * 32 + 16 + k:k * 32 + 16 + k + 1], 1.0)

        qw2 = cp.tile([128, M * 2 // 2], BF16)   # [(d,c), (t, mm, 2dup)]
        qpt = cp.tile([128, 4 * 64], F32)        # [m//64, (j, mi)]
        attn1 = cp.tile([128, 64 * S], F32)
        attn2 = cp.tile([128, 64 * S], BF16)

        qps_tiles = {}
        qps_fill = {}

        def phase1_chunk(cc):
            d, t = cc % 2, cc // 2
            if cc % 4 == 0 and cc >= 8:
                # next P quarter overwrites the oldest pool slot (SWDGE so the
                # trigger doesn't queue behind ScalarE copies)
                pch2 = pcp.tile([C, 4 * CH], BF16, tag="pch")
                nc.gpsimd.dma_start(pch2[:], P[:, cc * CH:(cc + 4) * CH])
                pts.append(pch2)
            pch = pts[cc // 4]
            rhs = pch[:, (cc % 4) * CH:(cc % 4 + 1) * CH]

            pq = psq.tile([128, CH], F32)
            nc.tensor.matmul(pq[:], lhsT=whs[d][:], rhs=rhs, start=True, stop=True)

            # qW rows -> qw2, duplicated into adjacent pairs (bf16)
            nc.scalar.copy(
                qw2[d * 64:d * 64 + 64, t * 2 * CH:(t + 1) * 2 * CH].rearrange(
                    "p (mm two) -> p mm two", two=2
                ),
                pq[d * 64:d * 64 + 64, :]
                .rearrange("p (mm one) -> p mm one", one=1)
                .broadcast_to((64, CH, 2)),
            )
            # qp rows -> staging, then scatter into qpt at each group of 4
            g, gi = cc // 4, cc % 4
            if g not in qps_tiles:
                qps_tiles[g] = qpsp.tile([4, 4 * CH], F32, name="qps", tag="qps")
                qps_fill[g] = 0
            qps = qps_tiles[g]
            nc.scalar.copy(
                qps[:, gi * CH:(gi + 1) * CH],
                pq[(1 - d) * 64:(1 - d) * 64 + 4, :],
            )
            qps_fill[g] += 1
            if qps_fill[g] == 4:
                for j in range(4):
                    nc.gpsimd.dma_start(
                        qpt[g * 32:(g + 1) * 32, j * 64:(j + 1) * 64],
                        qps[j:j + 1, :],
                    )
                del qps_tiles[g]

        def supertile(t):
            npt = npp.tile([128, MB * S], BF16)
            dma(npt[:], NP[:, t * MB * S:(t + 1) * MB * S])

            prod = prp.tile([128, MB * S], BF16)
            qwb = (
                qw2[:, t * 2 * MB:(t + 1) * 2 * MB]
                .rearrange("p (mm one two) -> p mm one two", one=1, two=2)
                .broadcast_to((128, MB, S // 2, 2))
            )
            nc.vector.tensor_mul(
                prod[:].rearrange("p (mm s2 two) -> p mm s2 two", s2=S // 2, two=2),
                npt[:].rearrange("p (mm s2 two) -> p mm s2 two", s2=S // 2, two=2),
                qwb,
            )

            ps = psm.tile([32, 512], F32)
            for k in range(16):
                nc.tensor.matmul(
                    ps[:],
                    lhsT=hs[:, k * 32:(k + 1) * 32],
                    rhs=prod[:, k * 512:(k + 1) * 512],
                    start=(k == 0),
                    stop=(k == 15),
                )
            sc = scp.tile([32, 512], F32)
            nc.scalar.copy(sc[:], ps[:])
            # row d*16+k = chunk k (mm in [32k, 32k+32)) of group d; lands at
            # attn1 partition 16t + 8d + k//2, cols (k%2)*512 + (mm%32)*16 + s.
            for d in range(2):
                nc.gpsimd.dma_start(
                    attn1[t * 16 + d * 8:t * 16 + d * 8 + 8, :].rearrange(
                        "p (k1 f) -> p k1 f", k1=2
                    ),
                    sc[d * 16:(d + 1) * 16, :],
                )

        ph2_state = {}

        def phase2a():
            # tmp[mi, j, s] = xyz[mi, j] - nx[mi, j, s]   (bf16, 2x packed)
            nx4 = nxt[:].rearrange(
                "p (mi j s2 two) -> p mi j s2 two", mi=64, j=3, s2=S // 2, two=2
            )
            xyzb = (
                xyz2[:]
                .rearrange("p (mi j one two) -> p mi j one two", mi=64, j=3, one=1, two=2)
                .broadcast_to((128, 64, 3, S // 2, 2))
            )
            tmp = p3072.tile([128, 64 * 3 * S], BF16, tag="big")
            tmp4 = tmp[:].rearrange(
                "p (mi j s2 two) -> p mi j s2 two", mi=64, j=3, s2=S // 2, two=2
            )
            nc.vector.tensor_sub(tmp4, xyzb, nx4)

            sq = p3072.tile([128, 64 * 3 * S], BF16, tag="big2")
            nc.scalar.square(sq[:], tmp[:])

            def jsl(tl, j):
                return tl[:].rearrange(
                    "p (mi j s2 two) -> p mi j s2 two", mi=64, j=3, s2=S // 2, two=2
                )[:, :, j, :, :]

            na = p1024.tile([128, 64 * S], BF16, tag="w1k")
            na3 = na[:].rearrange("p (mi s2 two) -> p mi s2 two", s2=S // 2, two=2)
            nc.vector.tensor_add(na3, jsl(sq, 0), jsl(sq, 1))
            norm2 = p1024.tile([128, 64 * S], BF16, tag="w1k")
            n23 = norm2[:].rearrange("p (mi s2 two) -> p mi s2 two", s2=S // 2, two=2)
            nc.vector.tensor_add(n23, na3, jsl(sq, 2))
            norm = p1024.tile([128, 64 * S], BF16, tag="w1k")
            nc.scalar.sqrt(norm[:], norm2[:])
            ph2_state["tmp"] = tmp
            ph2_state["norm"] = norm
            ph2_state["jsl"] = jsl

        def phase2b():
            # needs qpt complete (all 4 scatter groups) — emitted late so the
            # in-order DVE queue never waits on phase-1 tail mid-main-loop
            tmp = ph2_state["tmp"]
            norm = ph2_state["norm"]
            jsl = ph2_state["jsl"]

            # qpt2: bf16, duplicated pairs [p, (j, mi, 2)]
            qpt2 = sp.tile([128, 4 * 64 * 2], BF16)
            nc.vector.tensor_copy(
                qpt2[:].rearrange("p (j mi two) -> p j mi two", j=4, two=2),
                qpt[:]
                .rearrange("p (j mi one) -> p j mi one", j=4, one=1)
                .broadcast_to((128, 4, 64, 2)),
            )

            def qsl(j):
                return (
                    qpt2[:, j * 128:(j + 1) * 128]
                    .rearrange("p (mi one two) -> p mi one two", one=1, two=2)
                    .broadcast_to((128, 64, S // 2, 2))
                )

            # u = sum_j qp[j]*tmp[j]
            ua = p1024.tile([128, 64 * S], BF16, tag="w1k")
            ua3 = ua[:].rearrange("p (mi s2 two) -> p mi s2 two", s2=S // 2, two=2)
            nc.vector.tensor_mul(ua3, jsl(tmp, 0), qsl(0))
            ub = p1024.tile([128, 64 * S], BF16, tag="w1k")
            ub3 = ub[:].rearrange("p (mi s2 two) -> p mi s2 two", s2=S // 2, two=2)
            nc.vector.tensor_mul(ub3, jsl(tmp, 1), qsl(1))
            nc.vector.tensor_add(ua3, ua3, ub3)
            nc.vector.tensor_mul(ub3, jsl(tmp, 2), qsl(2))
            nc.vector.tensor_add(ua3, ua3, ub3)

            # attn2 = u + qp3 * norm
            a23 = ub3
            nc.vector.tensor_mul(
                a23,
                norm[:].rearrange("p (mi s2 two) -> p mi s2 two", s2=S // 2, two=2),
                qsl(3),
            )
            nc.vector.tensor_add(
                attn2[:].rearrange("p (mi s2 two) -> p mi s2 two", s2=S // 2, two=2),
                ua3,
                a23,
            )

        # ---- emission schedule: interleave phase 1 with supertiles so the
        # in-order engine queues never head-of-line block the main loop ----
        for t in range(ST):
            phase1_chunk(2 * t)
            phase1_chunk(2 * t + 1)
            if t == ST - 1:
                phase2b()
            supertile(t)
            if t == 2:
                phase2a()

        # ---- softmax over s ----
        attn = p1024.tile([128, 64 * S], F32, tag="w1kf")
        nc.vector.tensor_add(attn[:], attn1[:], attn2[:])
        at3 = attn[:].rearrange("p (mi s) -> p mi s", mi=64)

        mx = sp.tile([128, 64], F32)
        nc.vector.reduce_max(mx[:], at3, axis=AX.X)
        mxb = mx[:].rearrange("p (mi one) -> p mi one", one=1).broadcast_to((128, 64, S))
        xs = p1024.tile([128, 64 * S], F32, tag="w1kf")
        nc.vector.tensor_sub(xs[:].rearrange("p (mi s) -> p mi s", mi=64), at3, mxb)

        e = p1024.tile([128, 64 * S], F32, tag="w1kf")
        nc.scalar.activation(e[:], xs[:], AF.Exp)

        se = sp.tile([128, 64], F32)
        nc.vector.reduce_sum(se[:], e[:].rearrange("p (mi s) -> p mi s", mi=64), axis=AX.X)
        rse = sp.tile([128, 64], F32)
        nc.vector.reciprocal(rse[:], se[:])

        o = p1024.tile([128, 64 * S], F32, tag="w1kf")
        rb = rse[:].rearrange("p (mi one) -> p mi one", one=1).broadcast_to((128, 64, S))
        nc.vector.tensor_mul(
            o[:].rearrange("p (mi s) -> p mi s", mi=64),
            e[:].rearrange("p (mi s) -> p mi s", mi=64),
            rb,
        )
        dma(OUT, o[:])


_NC_CACHE = None


def build_nc():
    global _NC_CACHE
    if _NC_CACHE is None:
        nc = bacc.Bacc(trn_type="TRN2", target_bir_lowering=False, debug=False)
        with tile.TileContext(nc) as tc:
            _body(tc)
        nc.compile()
        _NC_CACHE = nc
    return _NC_CACHE


def make_in_maps(xyz, neighbor_xyz, points, neighbor_points, Wk, Wpos, bpos):
    """Slice + relayout + bf16-cast full inputs into the 8 per-core maps."""
    xyz = np.asarray(xyz, dtype=np.float32)
    neighbor_xyz = np.asarray(neighbor_xyz, dtype=np.float32)
    points = np.asarray(points, dtype=np.float32)
    neighbor_points = np.asarray(neighbor_points, dtype=np.float32)
    Wk = np.ascontiguousarray(np.asarray(Wk, dtype=np.float32))
    WkT = np.ascontiguousarray(Wk.T)
    Wp = np.ascontiguousarray(np.asarray(Wpos, dtype=np.float32))

    in_maps = []
    for i in range(NCORES):
        nsl = slice(i * NL, (i + 1) * NL)
        # np: [B,C,nl,S] -> [c, m, s] -> [(d,c), (t, mm, s)] bf16
        npc = (
            neighbor_points[:, :, nsl, :]
            .transpose(1, 0, 2, 3)
            .reshape(C, M, S)
            .astype(BF)
        )
        npc = (
            npc.reshape(C, ST, 2, MB, S)
            .transpose(2, 0, 1, 3, 4)
            .reshape(128, ST * MB * S)
        )
        # nx: [B,3,nl,S] -> [m, j, s] -> [128, (mi, j, s)] bf16
        nxc = (
            neighbor_xyz[:, :, nsl, :]
            .transpose(1, 0, 2, 3)
            .reshape(3, M, S)
            .transpose(1, 0, 2)
            .reshape(128, 64 * 3 * S)
            .astype(BF)
        )
        # xyz: [B,3,nl] -> [m, j] -> duplicated pairs [128, (mi, j, 2)] bf16
        xc = xyz[:, :, nsl].transpose(1, 0, 2).reshape(3, M).T.astype(BF)
        xc2 = np.repeat(xc, 2, axis=1).reshape(128, 64 * 3 * 2)
        # points: [B,C,nl] -> [c, m] bf16
        pc = points[:, :, nsl].transpose(1, 0, 2).reshape(C, M).astype(BF)
        in_maps.append(
            {
                "NP": np.ascontiguousarray(npc),
                "NX": np.ascontiguousarray(nxc),
                "XYZ2": np.ascontiguousarray(xc2),
                "P": np.ascontiguousarray(pc),
                "WK": Wk,
                "WKT": WkT,
                "WP": Wp,
            }
        )
    return in_maps


def assemble_output(results):
    """Per-core OUT [128, 64*S] -> full [B, N, S]."""
    out = np.empty((B, N, S), dtype=np.float32)
    for i in range(NCORES):
        oc = np.asarray(results[i]["OUT"]).reshape(M, S)  # m = p*64+mi row-major
        out[:, i * NL:(i + 1) * NL, :] = oc.reshape(B, NL, S)
    return out


def run_cores(in_maps, trace=False, trace_kwargs=None):
    nc = build_nc()
    return run_bass_kernel_spmd(
        nc,
        in_maps,
        core_ids=list(range(NCORES)),
        trace=trace,
        **(trace_kwargs or {}),
    )


def kernel(xyz, neighbor_xyz, points, neighbor_points, Wk, Wpos, bpos):
    in_maps = make_in_maps(
        xyz, neighbor_xyz, points, neighbor_points, Wk, Wpos, bpos
    )
    res = run_cores(in_maps, trace=False)
    return assemble_output(res.results)
